# revision 27
# baseline (speedup 1.0000x reference)
"""CAREConv forward kernel for Trainium2 (8 NeuronCores, Bass/Tile), v2.3.

Math (per node i with D=32 in-edges grouped by destination):
    t = tanh(feature @ W_mlp.T + b_mlp)            # [N, 2]
    d[i, j] = |t[src[i,j]] - t[i]|.sum()           # L1 dist, [N, D]
    keep K=16 smallest-d in-edges (ties -> lower j, matching lax.top_k)
    h_et[i] = mean_k feature[src[i, keep_k]]       # [N, F]
    out = (0.5 * h_et + feature) @ W_lin.T + b_lin # [N, H]

Design ("gather-all-32"): destination nodes sharded over the 8 cores
(12544 each after padding 100000 -> 100352).  The Q7 SWDGE descriptor
rate (~1.1us per 128-offset indirect DMA, ~8.6ns/row -- measured, and
identical for dma_gather at ~8ns/idx) is the hard bottleneck on this
problem, so the kernel does exactly ONE gather pass: per 128-node tile,
32 per-offset indirect DMAs fetch all 32 neighbor rows (bf16, 256B) of
the tile's nodes -- 401K descriptors/core instead of the baseline's
602K (t-gather + selected-feature gather).  Everything else hides under
that stream:
  * t for the sources is recomputed in-tile on the tensor engine
    (32 transposes -> GT, a [2 x 4096] W_mlp matmul, 32 tiny transposes
    back, fused tanh) -- no AllGather, no t table, no extra HBM traffic;
  * the 16-of-32 selection (DVE max8+match_replace on -d, two rounds,
    stable ties matching lax.top_k) produces a 0/1 mask that is applied
    as 32 accumulating matmuls sum_m G_m.T @ diag(mask_m), yielding
    h_et directly in [feature, node] layout for the fused residual +
    output GEMM.  No second gather, no DVE tree-sum.
bf16 features keep SBUF/DMA volume down; rel L2 err ~6e-3 (gate 2e-2).
"""

import numpy as np

import concourse.bacc as bacc
import concourse.bass as bass
import concourse.tile as tile
from concourse import mybir
from concourse.bass import IndirectOffsetOnAxis
from concourse.bass_utils import run_bass_kernel_spmd
from concourse.masks import make_identity

F32 = mybir.dt.float32
I32 = mybir.dt.int32
BF16 = mybir.dt.bfloat16

# Problem constants (hardcoded per harness contract).
N = 100_000      # real nodes
D = 32           # in-degree
K = 16           # neighbors kept (ceil(D * 0.5))
F = 128          # IN_FEATS
H = 64           # H_FEATS
C = 2            # NUM_CLASSES (t width)
PKEEP = 0.5
NCORES = 8
P = 128          # partitions
SHARD = 12_544   # nodes per core (padded)
NPAD = SHARD * NCORES  # 100352

MINVAL = float(-(2 ** 30))


def build(npad=NPAD, shard=SHARD, ncores=NCORES):
    tiles = shard // P
    assert shard % P == 0 and npad == shard * ncores

    nc = bacc.Bacc("TRN2", target_bir_lowering=False, debug=False,
                   num_devices=ncores, dynamic_dma_scratch_size=65536)

    feat_bf = nc.dram_tensor("feat_bf", [npad, F], BF16, kind="ExternalInput")
    feat_own = nc.dram_tensor("feat_own", [shard, F], F32,
                              kind="ExternalInput")
    src_own = nc.dram_tensor("src_own", [shard, D], I32, kind="ExternalInput")
    w_mlp_t = nc.dram_tensor("w_mlp_t", [F, C], F32, kind="ExternalInput")
    w_mlp_b = nc.dram_tensor("w_mlp_b", [F, C], BF16, kind="ExternalInput")
    b_mlp = nc.dram_tensor("b_mlp", [C, 1], F32, kind="ExternalInput")
    w_lin_t = nc.dram_tensor("w_lin_t", [F, H], F32, kind="ExternalInput")
    b_lin = nc.dram_tensor("b_lin", [H, 1], F32, kind="ExternalInput")
    out_t = nc.dram_tensor("out_t", [H, shard], F32, kind="ExternalOutput")

    ts = bass.ts

    with tile.TileContext(nc) as tc:
        with (
            tc.tile_pool(name="const", bufs=1) as cpool,
            tc.tile_pool(name="persist", bufs=1) as ppool,
        ):
            ident = cpool.tile([P, P], F32)
            make_identity(nc, ident[:])
            identb = cpool.tile([P, P], BF16)
            nc.vector.tensor_copy(identb[:], ident[:])
            wm = cpool.tile([F, C], F32)
            nc.sync.dma_start(wm[:], w_mlp_t[:, :])
            wmb = cpool.tile([F, C], BF16)
            nc.sync.dma_start(wmb[:], w_mlp_b[:, :])
            wl = cpool.tile([F, H], F32)
            nc.sync.dma_start(wl[:], w_lin_t[:, :])
            bm = cpool.tile([C, 1], F32)
            nc.sync.dma_start(bm[:], b_mlp[:, :])
            bl = cpool.tile([H, 1], F32)
            nc.sync.dma_start(bl[:], b_lin[:, :])

            # Persistent SBUF: transposed own features + negated own t.
            featT = ppool.tile([P, tiles * P], F32)     # [feat, own nodes]
            tneg = ppool.tile([P, tiles * C], F32)      # -t_own per tile

            # ---------------- Phase 1: own-node t + featT -------------------
            with (
                tc.tile_pool(name="p1", bufs=3) as p1,
                tc.tile_pool(name="p1ps", bufs=2, space="PSUM") as p1ps,
            ):
                SB1 = 7
                for ib in range(0, tiles, SB1):
                    nb = min(SB1, tiles - ib)
                    ftb = p1.tile([P, SB1, F], F32, tag="ftb")
                    nc.gpsimd.dma_start(
                        ftb[:, 0:nb, :],
                        bass.AP(feat_own, ib * P * F,
                                [[F, P], [P * F, nb], [1, F]]))
                    for k in range(nb):
                        i = ib + k
                        ps_tr = p1ps.tile([P, P], F32, tag="ps_tr")
                        nc.tensor.transpose(ps_tr[:], ftb[:, k, :], ident[:])
                        nc.scalar.copy(featT[:, ts(i, P)], ps_tr[:])
                        ps_z = p1ps.tile([C, P], F32, tag="ps_z")
                        nc.tensor.matmul(out=ps_z[:], lhsT=wm[:],
                                         rhs=featT[:, ts(i, P)],
                                         start=True, stop=True)
                        tk = p1.tile([C, P], F32, tag="tk")
                        nc.scalar.activation(
                            tk[:], ps_z[:],
                            mybir.ActivationFunctionType.Tanh,
                            bias=bm[:, 0:1])
                        ps_to = p1ps.tile([P, C], F32, tag="ps_to")
                        nc.tensor.transpose(ps_to[:], tk[:], ident[:C, :C])
                        nc.scalar.mul(tneg[:, ts(i, C)], ps_to[:], -1.0)

            # ---------------- Phase 2: gather-all-32 per tile ---------------
            with (
                tc.tile_pool(name="p2", bufs=2) as p2,
                tc.tile_pool(name="p2g", bufs=4) as p2g,
                tc.tile_pool(name="p2s", bufs=3) as p2s,
                tc.tile_pool(name="p2ps", bufs=2, space="PSUM") as p2ps,
                tc.tile_pool(name="p2ph", bufs=1, space="PSUM") as p2ph,
            ):
                SB = 4  # tiles per sidx batch load
                for ib in range(0, tiles, SB):
                    nb = min(SB, tiles - ib)
                    sidx = p2s.tile([P, nb * D], I32, tag="sidx")
                    nc.sync.dma_start(
                        sidx[:],
                        bass.AP(src_own, ib * P * D,
                                [[D, P], [P * D, nb], [1, D]]))

                    for k2 in range(nb):
                        i = ib + k2
                        # -- 32 per-offset indirect gathers: G [p, m, f] bf16
                        G = p2g.tile([P, D, F], BF16, tag="G")
                        for m in range(D):
                            nc.gpsimd.indirect_dma_start(
                                out=G[:, m, :], out_offset=None,
                                in_=feat_bf[:, :],
                                in_offset=IndirectOffsetOnAxis(
                                    ap=sidx[:, k2 * D + m:k2 * D + m + 1],
                                    axis=0))

                        # -- transpose each G_m -> GT [f, (m p)] bf16 in SBUF
                        GT = p2.tile([P, D * P], BF16, tag="GT")
                        for m in range(D):
                            ps_g = p2ps.tile([P, P], BF16, tag="ps_g")
                            nc.tensor.transpose(ps_g[:], G[:, m, :], identb[:])
                            if m % 2 == 0:
                                nc.scalar.copy(GT[:, ts(m, P)], ps_g[:])
                            else:
                                nc.vector.tensor_copy(GT[:, ts(m, P)], ps_g[:])

                        # -- z = Wmlp @ f_src for all 4096 edges: [2, (m p)]
                        zsb = p2.tile([C, D * P], F32, tag="zsb")
                        for zb in range(8):
                            ps_zz = p2ps.tile([C, 512], F32, tag="ps_zz")
                            nc.tensor.matmul(out=ps_zz[:], lhsT=wmb[:],
                                             rhs=GT[:, ts(zb, 512)],
                                             start=True, stop=True)
                            # + b_mlp while components are on partitions
                            nc.vector.tensor_scalar(
                                zsb[:, ts(zb, 512)], ps_zz[:], bm[:, 0:1],
                                None, op0=mybir.AluOpType.add)

                        # -- transpose z blocks to [p, (m c)] + fused tanh
                        ps_zt = p2ps.tile([P, D * C], F32, tag="ps_zt")
                        for m in range(D):
                            nc.tensor.transpose(
                                ps_zt[:, ts(m, C)], zsb[:, ts(m, P)],
                                ident[:C, :C])
                        tsrc = p2.tile([P, D * C], F32, tag="tsrc")
                        nc.scalar.activation(tsrc[:], ps_zt[:],
                                             mybir.ActivationFunctionType.Tanh,
                                             bias=0.0)
                        tv = tsrc[:].rearrange("p (m c) -> p m c", c=C)

                        # -- d = |ta_src - ta_own| + |tb_src - tb_own|
                        absa = p2s.tile([P, D], F32, tag="absa")
                        nc.scalar.activation(
                            absa[:], tv[:, :, 0],
                            mybir.ActivationFunctionType.Abs,
                            bias=tneg[:, i * C:i * C + 1])
                        absb = p2s.tile([P, D], F32, tag="absb")
                        nc.scalar.activation(
                            absb[:], tv[:, :, 1],
                            mybir.ActivationFunctionType.Abs,
                            bias=tneg[:, i * C + 1:i * C + 2])
                        negd = p2s.tile([P, D], F32, tag="negd")
                        nc.vector.scalar_tensor_tensor(
                            out=negd[:], in0=absa[:], scalar=-1.0,
                            in1=absb[:],
                            op0=mybir.AluOpType.mult,
                            op1=mybir.AluOpType.subtract)

                        # -- top-16 smallest d: two max8+match_replace rounds
                        v8a = p2s.tile([P, 8], F32, tag="v8a")
                        nc.vector.max(v8a[:], negd[:])
                        negd2 = p2s.tile([P, D], F32, tag="negd2")
                        nc.vector.match_replace(
                            out=negd2[:], in_to_replace=v8a[:],
                            in_values=negd[:], imm_value=MINVAL)
                        v8b = p2s.tile([P, 8], F32, tag="v8b")
                        nc.vector.max(v8b[:], negd2[:])
                        negd3 = p2s.tile([P, D], F32, tag="negd3")
                        nc.vector.match_replace(
                            out=negd3[:], in_to_replace=v8b[:],
                            in_values=negd2[:], imm_value=MINVAL)
                        mask = p2s.tile([P, D], F32, tag="mask")
                        nc.vector.tensor_scalar(
                            mask[:], negd3[:], MINVAL, None,
                            op0=mybir.AluOpType.is_equal)

                        # -- Dm = diag(mask[:, m]); 32 accumulating matmuls
                        Dm = p2.tile([P, D * P], BF16, tag="Dm")
                        for m in range(D):
                            nc.vector.tensor_scalar(
                                Dm[:, ts(m, P)], identb[:], mask[:, m:m + 1],
                                None, op0=mybir.AluOpType.mult)
                        ps_h = p2ph.tile([P, P], F32, tag="ps_h")
                        for m in range(D):
                            nc.tensor.matmul(
                                out=ps_h[:], lhsT=G[:, m, :],
                                rhs=Dm[:, ts(m, P)],
                                start=(m == 0), stop=(m == D - 1))

                        # -- hT = psum_h * (0.5/16) + featT ; out GEMM
                        hT = p2.tile([P, P], F32, tag="hT")
                        nc.vector.scalar_tensor_tensor(
                            out=hT[:], in0=ps_h[:], scalar=PKEEP / K,
                            in1=featT[:, ts(i, P)],
                            op0=mybir.AluOpType.mult,
                            op1=mybir.AluOpType.add)
                        ps_o = p2ph.tile([H, P], F32, tag="ps_o")
                        nc.tensor.matmul(out=ps_o[:], lhsT=wl[:], rhs=hT[:],
                                         start=True, stop=True)
                        ob = p2s.tile([H, P], F32, tag="ob")
                        nc.vector.tensor_scalar(
                            ob[:], ps_o[:], bl[:, 0:1], None,
                            op0=mybir.AluOpType.add)
                        nc.sync.dma_start(out_t[:, ts(i, P)], ob[:])

    nc.compile()
    return nc


_NC_CACHE = {}


def _get_nc():
    key = (NPAD, SHARD, NCORES)
    if key not in _NC_CACHE:
        _NC_CACHE[key] = build(NPAD, SHARD, NCORES)
    return _NC_CACHE[key]


def make_in_maps(feature, src_ids, W_mlp, b_mlp, W_lin, b_lin,
                 npad=NPAD, shard=SHARD, ncores=NCORES):
    import ml_dtypes

    n, f = feature.shape
    fpad = np.zeros((npad, f), np.float32)
    fpad[:n] = np.asarray(feature, np.float32)
    fbf = fpad.astype(ml_dtypes.bfloat16)
    spad = np.zeros((npad * D,), np.int32)
    spad[:src_ids.size] = np.asarray(src_ids, np.int32).ravel()
    src2d = spad.reshape(npad, D)
    wmt = np.ascontiguousarray(np.asarray(W_mlp, np.float32).T)
    wmb = wmt.astype(ml_dtypes.bfloat16)
    wlt = np.ascontiguousarray(np.asarray(W_lin, np.float32).T)
    bm = np.asarray(b_mlp, np.float32).reshape(C, 1)
    bl = np.asarray(b_lin, np.float32).reshape(H, 1)
    in_maps = []
    for c in range(ncores):
        sl = slice(c * shard, (c + 1) * shard)
        in_maps.append({
            "feat_bf": fbf,
            "feat_own": np.ascontiguousarray(fpad[sl]),
            "src_own": np.ascontiguousarray(src2d[sl]),
            "w_mlp_t": wmt,
            "w_mlp_b": wmb,
            "b_mlp": bm,
            "w_lin_t": wlt,
            "b_lin": bl,
        })
    return in_maps


def run(feature, src_ids, W_mlp, b_mlp, W_lin, b_lin, **spmd_kwargs):
    """Run on hardware; returns (output [N, H] f32, BassKernelResults)."""
    nc = _get_nc()
    in_maps = make_in_maps(feature, src_ids, W_mlp, b_mlp, W_lin, b_lin)
    res = run_bass_kernel_spmd(nc, in_maps, core_ids=list(range(NCORES)),
                               **spmd_kwargs)
    outs = [res.results[c]["out_t"] for c in range(NCORES)]
    full = np.concatenate([o.T for o in outs], axis=0)[:N]
    return np.ascontiguousarray(full, dtype=np.float32), res


def kernel(feature, src_ids, W_mlp, b_mlp, W_lin, b_lin):
    out, _ = run(feature, src_ids, W_mlp, b_mlp, W_lin, b_lin)
    return out
